# revision 16
# baseline (speedup 1.0000x reference)
"""AdaptiveQuadratureHead Trainium2 kernel.

8-core SPMD, data-parallel over batch (B=8 -> one batch element per core).
No collectives. Host marshals inputs into feature-major bf16 layouts with a
virtual token order (physical token q*64+t -> tile t, partition q) so that
per-token scalars (quadrature weights) land as contiguous [128, 1] columns.

Math (per core, N=8192 tokens):
  K = gelu(gelu(xu @ W1k + kb1) @ kW2 + kb2) @ kW3 + kb3       (feature-major)
  V = gelu(gelu(xu @ W1v + vb1) @ vW2 + vb2) @ vW3 (token-major, h2-stationary)
  w = max(sensor_weights, 0) * mask ; denom = max(sum w, eps)
  c = (sum_t w*V)/denom + vb3
  a = tanh(gelu(c@aW1+ab1) @ aW2 + ab2) ; Q^T = qT + (0.1*B_dirs)^T a^T
  scoresT = K^T Q ; Phi = softplus(scoresT/8) = ln(exp(scoresT/8)+1)
  pooledT = (V*w | w)^T-accumulated Phi matmuls; /denom; +vb3 correction
  out = gelu(pooledT^T rW1 + rb1) @ rW2 + rb2
"""

import sys
import types
from contextlib import ExitStack

import numpy as np
import ml_dtypes

B, N = 8, 8192
DX, DU = 64, 4
P, DK, DV, DOUT = 128, 64, 64, 128
H, R, AH = 256, 4, 64
ADAPT_SCALE = 0.1
EPS = 1e-8
NT = 64          # token tiles of 128 (virtual order)
NCH = 16         # column chunks of 512
VW_W = 65        # V*w tile width (64 features + w column)
NG = 8           # score groups (8 tiles of 128 -> [128, 1024])
BF16 = ml_dtypes.bfloat16

# bf16 weight pack column offsets (rows 0-127)
WB_COLS = {
    "kW1": (0, 256), "vW1": (256, 512),
    "kW2a": (512, 768), "kW2b": (768, 1024),
    "vW2a": (1024, 1280), "vW2b": (1280, 1536),
    "kW3a": (1536, 1600), "kW3b": (1600, 1664),
    "vW3a": (1664, 1728), "vW3b": (1728, 1792),
    "rW2a": (1792, 1920), "rW2b": (1920, 2048),
    "rW1": (2048, 2304), "rb2": (2304, 2432),
}
WB_W = 2432
# f32 pack A [128, 138]
FA_COLS = {"sw": (0, 64), "mk": (64, 128), "kb1": (128, 130),
           "vb1": (130, 132), "kb2": (132, 134), "vb2": (134, 136),
           "rb1": (136, 138)}
FA_W = 138
# f32 pack B [64, 1283]
FB_COLS = {"qT": (0, 128), "aW2": (128, 640), "aW1": (640, 704),
           "ab1": (704, 705), "kb3": (705, 706), "vb3": (706, 707),
           "Bd": (707, 771), "ab2": (771, 1283)}
FB_W = 1283

_CACHE = {}


def _install_ntff_hook_shim():
    """This image's antenv lacks axon_hooks; provide it so
    run_bass_kernel_spmd(trace=True) can reach the ctypes NTFF hook."""
    if "antenv.axon_hooks" in sys.modules:
        return
    try:
        from trn_agent_boot.trn_boot import _ntff_profile_via_ctypes
        hook = _ntff_profile_via_ctypes("/opt/axon/libaxon_pjrt.so")
    except Exception:
        hook = None
    mod = types.ModuleType("antenv.axon_hooks")
    mod._hook = hook
    mod.get_axon_ntff_profile_hook = lambda: mod._hook
    mod.set_axon_ntff_profile_hook = lambda h: setattr(mod, "_hook", h)
    sys.modules["antenv.axon_hooks"] = mod
    try:
        import antenv
        antenv.axon_hooks = mod
    except Exception:
        pass


def _build(biases_zero: bool, vb3_zero: bool, ab2_zero: bool, rb2_zero: bool):
    import concourse.bass as bass
    import concourse.bacc as bacc
    import concourse.mybir as mybir
    import concourse.tile as tile
    from concourse.tile_rust import add_dep_helper

    AF = mybir.ActivationFunctionType
    OP = mybir.AluOpType
    AX = mybir.AxisListType
    f32 = mybir.dt.float32
    bf16 = mybir.dt.bfloat16

    nc = bacc.Bacc(None, target_bir_lowering=False)

    def din(name, shape, dt):
        return nc.declare_dram_parameter(name, list(shape), dt, isOutput=False)

    xu_d = din("xu_fm", (68, N), bf16)
    wb_d = din("wb16", (128, WB_W), bf16)
    fa_d = din("f32a", (128, FA_W), f32)
    fb_d = din("f32b", (64, FB_W), f32)
    out_d = nc.declare_dram_parameter("out", [P, DOUT], f32, isOutput=True)

    with tile.TileContext(nc) as tc, ExitStack() as ctx:
        const = ctx.enter_context(tc.tile_pool(name="const", bufs=1))
        xu_pool = ctx.enter_context(tc.tile_pool(name="xu", bufs=NCH))
        h_pool = ctx.enter_context(tc.tile_pool(name="hsb", bufs=6))
        phi_pool = ctx.enter_context(tc.tile_pool(name="phi", bufs=2))
        es_pool = ctx.enter_context(tc.tile_pool(name="es", bufs=NG))
        ps_big = ctx.enter_context(
            tc.tile_pool(name="psb", bufs=3, space=bass.MemorySpace.PSUM))
        ps_sm = ctx.enter_context(
            tc.tile_pool(name="pss", bufs=2, space=bass.MemorySpace.PSUM))

        # ---- packed constants: few big DMAs, spread across dispatchers ----
        wb_t = const.tile([128, WB_W], bf16, tag="wb")
        nc.gpsimd.dma_start(wb_t[:, 0:256], wb_d[:, 0:256])
        nc.gpsimd.dma_start(wb_t[:, 256:512], wb_d[:, 256:512])
        for c0, c1 in ((512, 1024), (1024, 1536), (1536, 2048),
                       (2048, WB_W)):
            nc.gpsimd.dma_start(wb_t[:, c0:c1], wb_d[:, c0:c1])
        fa_t = const.tile([128, FA_W], f32, tag="fa")
        nc.gpsimd.dma_start(fa_t[:], fa_d[:])
        fb_t = const.tile([64, FB_W], f32, tag="fb")
        nc.gpsimd.dma_start(fb_t[:], fb_d[:])

        def wb(name, rows=128):
            c0, c1 = WB_COLS[name]
            return wb_t[0:rows, c0:c1]

        def fa(name):
            c0, c1 = FA_COLS[name]
            return fa_t[:, c0:c1]

        def fb(name, rows=64):
            c0, c1 = FB_COLS[name]
            return fb_t[0:rows, c0:c1]

        K_fm = const.tile([DK, N], bf16, tag="K_fm")
        Vw_all = const.tile([128, NT * VW_W], bf16, tag="Vw")
        Vacc = const.tile([128, 4 * DV], f32, tag="Vacc")
        w_t = const.tile([128, NT], f32, tag="w")
        ones_col = const.tile([128, 1], f32, tag="ones_c")
        ones_r64 = const.tile([1, 64], f32, tag="ones_r")
        ones_rP = const.tile([1, P], bf16, tag="ones_p")

        nc.gpsimd.memset(Vacc[:], 0.0)
        nc.vector.memset(ones_col[:], 1.0)
        nc.vector.memset(ones_r64[:], 1.0)
        nc.vector.memset(ones_rP[:], 1.0)

        # quadrature weights: w = max(sw, 0) * mask
        nc.vector.tensor_scalar(w_t[:], fa("sw"), 0.0, None, OP.max)
        nc.vector.tensor_tensor(w_t[:], w_t[:], fa("mk"), op=OP.mult)

        Vw_v = Vw_all[:].rearrange("p (t c) -> p t c", c=VW_W)

        # ========== PASS A: staggered V-net(i) / K-net(i-2) ==========
        # V feeds the global c reduction, K only feeds scores; lagging the
        # K-net keeps ACT saturated while letting pass B's serial chain
        # hide under the last K-net chunks.
        xu_tiles = []

        def v_net(j):
            cs = slice(j * 512, (j + 1) * 512)
            xu = xu_pool.tile([68, 512], bf16, tag="xu")
            nc.sync.dma_start(xu[:], xu_d[:, cs])
            xu_tiles.append(xu)

            h1V_ps = ps_big.tile([128, 1024], f32, tag="ps")
            nc.tensor.matmul(h1V_ps[:, 0:512], wb("vW1", 68)[:, 0:128], xu[:])
            nc.tensor.matmul(h1V_ps[:, 512:1024], wb("vW1", 68)[:, 128:256],
                             xu[:])
            h1V = h_pool.tile([128, 1024], bf16, tag="h")
            if biases_zero:
                nc.scalar.activation(h1V[:], h1V_ps[:], AF.Gelu)
            else:
                for ho in range(2):
                    hs = slice(ho * 512, (ho + 1) * 512)
                    nc.scalar.activation(h1V[:, hs], h1V_ps[:, hs], AF.Gelu,
                                         bias=fa("vb1")[:, ho:ho + 1])

            h2V_ps = ps_big.tile([128, 1024], f32, tag="ps")
            for ho in range(2):
                hs = slice(ho * 512, (ho + 1) * 512)
                wsl = slice(ho * 128, (ho + 1) * 128)
                nc.tensor.matmul(h2V_ps[:, hs], wb("vW2a")[:, wsl],
                                 h1V[:, 0:512], start=True, stop=False)
                nc.tensor.matmul(h2V_ps[:, hs], wb("vW2b")[:, wsl],
                                 h1V[:, 512:1024], start=False, stop=True)
            h2V = h_pool.tile([128, 1024], bf16, tag="h")
            if biases_zero:
                nc.scalar.activation(h2V[:], h2V_ps[:], AF.Gelu)
            else:
                for ho in range(2):
                    hs = slice(ho * 512, (ho + 1) * 512)
                    nc.scalar.activation(h2V[:, hs], h2V_ps[:, hs], AF.Gelu,
                                         bias=fa("vb2")[:, ho:ho + 1])

            v_ps = ps_sm.tile([128, 4 * DV], f32, tag="ps")
            for st in range(4):
                vs = slice(st * DV, (st + 1) * DV)
                ts_a = slice(st * 128, (st + 1) * 128)
                ts_b = slice(512 + st * 128, 512 + (st + 1) * 128)
                nc.tensor.matmul(v_ps[:, vs], h2V[:, ts_a], wb("vW3a"),
                                 start=True, stop=False)
                nc.tensor.matmul(v_ps[:, vs], h2V[:, ts_b], wb("vW3b"),
                                 start=False, stop=True)

            tsl = slice(4 * j, 4 * j + 4)
            w_b = w_t[:, tsl].to_broadcast([128, 4, DV])
            v4 = v_ps[:].rearrange("p (t c) -> p t c", c=DV)
            nc.vector.tensor_tensor(Vw_v[:, tsl, 0:DV], v4, w_b, op=OP.mult)
            if not vb3_zero:
                nc.vector.tensor_copy(
                    Vw_v[:, tsl, DV:VW_W],
                    w_t[:, tsl].rearrange("p (t c) -> p t c", c=1))
            nc.vector.tensor_tensor(Vacc[:], Vacc[:], Vw_v[:, tsl, 0:DV],
                                    op=OP.add)

        def k_net(j):
            cs = slice(j * 512, (j + 1) * 512)
            xu = xu_tiles[j]
            h1K_ps = ps_big.tile([128, 1024], f32, tag="ps")
            nc.tensor.matmul(h1K_ps[:, 0:512], wb("kW1", 68)[:, 0:128], xu[:])
            nc.tensor.matmul(h1K_ps[:, 512:1024], wb("kW1", 68)[:, 128:256],
                             xu[:])
            h1K = h_pool.tile([128, 1024], bf16, tag="h")
            if biases_zero:
                nc.scalar.activation(h1K[:], h1K_ps[:], AF.Gelu)
            else:
                for ho in range(2):
                    hs = slice(ho * 512, (ho + 1) * 512)
                    nc.scalar.activation(h1K[:, hs], h1K_ps[:, hs], AF.Gelu,
                                         bias=fa("kb1")[:, ho:ho + 1])

            h2K_ps = ps_big.tile([128, 1024], f32, tag="ps")
            for ho in range(2):
                hs = slice(ho * 512, (ho + 1) * 512)
                wsl = slice(ho * 128, (ho + 1) * 128)
                nc.tensor.matmul(h2K_ps[:, hs], wb("kW2a")[:, wsl],
                                 h1K[:, 0:512], start=True, stop=False)
                nc.tensor.matmul(h2K_ps[:, hs], wb("kW2b")[:, wsl],
                                 h1K[:, 512:1024], start=False, stop=True)
            h2K = h_pool.tile([128, 1024], bf16, tag="h")
            if biases_zero:
                nc.scalar.activation(h2K[:], h2K_ps[:], AF.Gelu)
            else:
                for ho in range(2):
                    hs = slice(ho * 512, (ho + 1) * 512)
                    nc.scalar.activation(h2K[:, hs], h2K_ps[:, hs], AF.Gelu,
                                         bias=fa("kb2")[:, ho:ho + 1])

            k_ps = ps_sm.tile([DK, 512], f32, tag="ps")
            nc.tensor.matmul(k_ps[:], wb("kW3a"), h2K[:, 0:512],
                             start=True, stop=False)
            nc.tensor.matmul(k_ps[:], wb("kW3b"), h2K[:, 512:1024],
                             start=False, stop=True)
            nc.vector.tensor_scalar_add(K_fm[:, cs], k_ps[:], fb("kb3"))

        LAG = 2
        for i in range(NCH):
            v_net(i)
            if i >= LAG:
                k_net(i - LAG)

        # ======= PASS B: c, denom, adaptive query (under A2's shadow) =======
        wsum = const.tile([128, 1], f32, tag="wsum")
        nc.vector.reduce_sum(wsum[:], w_t[:], axis=AX.X)
        den_ps = ps_sm.tile([1, 1], f32, tag="ps")
        nc.tensor.matmul(den_ps[:], wsum[:], ones_col[:])
        den_t = const.tile([1, 1], f32, tag="den")
        nc.vector.tensor_scalar(den_t[:], den_ps[:], EPS, None, OP.max)
        rec_t = const.tile([1, 1], f32, tag="rec")
        nc.vector.reciprocal(rec_t[:], den_t[:])
        recb_ps = ps_sm.tile([64, 1], f32, tag="ps")
        nc.tensor.matmul(recb_ps[:], ones_r64[:], rec_t[:])
        recb_t = const.tile([64, 1], f32, tag="recb")
        nc.vector.tensor_copy(recb_t[:], recb_ps[:])

        Vred = const.tile([128, DV], f32, tag="Vred")
        nc.vector.reduce_sum(
            Vred[:], Vacc[:].rearrange("p (t c) -> p c t", c=DV), axis=AX.X)
        c_ps = ps_sm.tile([DV, 1], f32, tag="ps")
        nc.tensor.matmul(c_ps[:], Vred[:], ones_col[:])

        # c = c_raw/denom + vb3   (max(denom,eps)=denom whenever w-sum > eps)
        c_t = const.tile([DV, 1], f32, tag="c")
        nc.vector.scalar_tensor_tensor(c_t[:], c_ps[:], recb_t[:], fb("vb3"),
                                       op0=OP.mult, op1=OP.add)

        # adaptive query: g = gelu(aW1.T c + ab1); a = tanh(aW2.T g + ab2)
        g_ps = ps_sm.tile([AH, 1], f32, tag="ps")
        nc.tensor.matmul(g_ps[:], fb("aW1"), c_t[:])
        g_t = const.tile([AH, 1], f32, tag="g")
        nc.scalar.activation(g_t[:], g_ps[:], AF.Gelu, bias=fb("ab1"))
        a_ps = ps_sm.tile([1, P * R], f32, tag="ps")
        nc.tensor.matmul(a_ps[:], g_t[:], fb("aW2"))
        av_t = const.tile([1, P * R], f32, tag="av")
        if ab2_zero:
            nc.scalar.activation(av_t[:], a_ps[:], AF.Tanh)
        else:
            a1_t = const.tile([1, P * R], f32, tag="a1")
            nc.vector.tensor_tensor(a1_t[:], a_ps[:], fb("ab2", 1), op=OP.add)
            nc.scalar.activation(av_t[:], a1_t[:], AF.Tanh)
        aT_t = const.tile([R, P], f32, tag="aT")
        av_v = av_t[0:1, :].rearrange("o (p r) -> o r p", r=R)
        for r, eng in enumerate((nc.sync, nc.scalar, nc.gpsimd, nc.sync)):
            eng.dma_start(aT_t[r:r + 1, :], av_v[:, r, :])
        qd_ps = ps_sm.tile([DK, P], f32, tag="ps")
        nc.tensor.matmul(qd_ps[:], fb("Bd", R), aT_t[:])
        QT_t = const.tile([DK, P], bf16, tag="QT")
        nc.vector.tensor_tensor(QT_t[:], qd_ps[:], fb("qT"), op=OP.add)

        # ========== PASS A tail: remaining K-net chunks ==========
        for j in range(NCH - LAG, NCH):
            k_net(j)

        # ================= PASS C: scores + pooled =================
        # softplus(s/sqrt(dk)) = ln(exp(s/8) + 1); scores scale ~0.1 so exp
        # cannot overflow. No native softplus table in this toolchain, and
        # the table chooser puts Exp and Ln in different sets, so run ALL
        # exps then ALL lns (ordering-pinned) to avoid table-load thrash.
        pool_m = DV if vb3_zero else VW_W
        pool_ps = ps_sm.tile([pool_m, P], f32, tag="ps")
        es_tiles = []
        last_exp = None
        for g in range(NG):
            sc_ps = ps_big.tile([128, 1024], f32, tag="ps")
            for s in range(8):
                t = g * 8 + s
                nc.tensor.matmul(sc_ps[:, s * 128:(s + 1) * 128],
                                 K_fm[:, t * 128:(t + 1) * 128], QT_t[:])
            es = es_pool.tile([128, 1024], f32, tag="es")
            last_exp = nc.scalar.activation(es[:], sc_ps[:], AF.Exp,
                                            scale=float(1.0 / np.sqrt(DK)))
            es_tiles.append(es)
        for g in range(NG):
            phi = phi_pool.tile([128, 1024], bf16, tag="phi")
            ln_i = nc.scalar.activation(phi[:], es_tiles[g][:], AF.Ln,
                                        bias=1.0)
            add_dep_helper(ln_i.ins, last_exp.ins, sync=False,
                           reason="batch act-table sets: all exps before lns")
            for s in range(8):
                t = g * 8 + s
                nc.tensor.matmul(pool_ps[:], Vw_v[:, t, 0:pool_m],
                                 phi[:, s * 128:(s + 1) * 128],
                                 start=(t == 0), stop=(t == NT - 1))

        # ================= PASS D: normalize + rho MLP =================
        poolb_t = const.tile([DV, P], bf16, tag="poolb")
        if vb3_zero:
            nc.vector.tensor_scalar_mul(poolb_t[:], pool_ps[0:DV, :],
                                        recb_t[:])
        else:
            swp_t = const.tile([1, P], f32, tag="swp")
            nc.vector.tensor_scalar_mul(swp_t[:], pool_ps[DV:VW_W, :],
                                        rec_t[:])
            swpb_ps = ps_sm.tile([DV, P], f32, tag="ps")
            nc.tensor.matmul(swpb_ps[:], ones_r64[:], swp_t[:])
            pooln_t = const.tile([DV, P], f32, tag="pooln")
            nc.vector.tensor_scalar_mul(pooln_t[:], pool_ps[0:DV, :],
                                        recb_t[:])
            nc.vector.scalar_tensor_tensor(poolb_t[:], swpb_ps[:], fb("vb3"),
                                           pooln_t[:], op0=OP.mult,
                                           op1=OP.add)

        # rho L1: hr = gelu(rW1.T pooledT + rb1)
        hr_sb = []
        for hc in range(2):
            hr_ps = ps_sm.tile([128, P], f32, tag="ps")
            nc.tensor.matmul(hr_ps[:],
                             wb("rW1", 64)[:, hc * 128:(hc + 1) * 128],
                             poolb_t[:])
            hr = const.tile([128, P], bf16, tag=f"hr{hc}")
            nc.scalar.activation(hr[:], hr_ps[:], AF.Gelu,
                                 bias=fa("rb1")[:, hc:hc + 1])
            hr_sb.append(hr)

        # rho L2: out = hr.T rW2 + rb2  (bias via rank-1 ones matmul)
        o_ps = ps_sm.tile([P, DOUT], f32, tag="ps")
        nc.tensor.matmul(o_ps[:], hr_sb[0][:], wb("rW2a"),
                         start=True, stop=False)
        nc.tensor.matmul(o_ps[:], hr_sb[1][:], wb("rW2b"),
                         start=False, stop=rb2_zero)
        if not rb2_zero:
            nc.tensor.matmul(o_ps[:], ones_rP[:], wb("rb2", 1),
                             start=False, stop=True)
        o_sb = const.tile([P, DOUT], f32, tag="osb")
        nc.vector.tensor_copy(o_sb[:], o_ps[:])
        nc.sync.dma_start(out_d[:], o_sb[:])

    nc.compile()
    return nc


def _prep_maps(inputs):
    f32 = np.float32
    x_enc = np.asarray(inputs["x_enc"], f32)
    u = np.asarray(inputs["u"], f32)
    mask = np.asarray(inputs["sensor_mask"]).astype(f32)
    sw = np.asarray(inputs["sensor_weights"], f32)

    g = {k: np.asarray(inputs[k], f32) for k in
         ("kW1", "kb1", "kW2", "kb2", "kW3", "kb3",
          "vW1", "vb1", "vW2", "vb2", "vW3", "vb3",
          "query_tokens", "B_dirs", "aW1", "ab1", "aW2", "ab2",
          "rW1", "rb1", "rW2", "rb2")}

    biases_zero = bool(all(np.all(g[k] == 0)
                           for k in ("kb1", "vb1", "kb2", "vb2")))
    vb3_zero = bool(np.all(g["vb3"] == 0))
    ab2_zero = bool(np.all(g["ab2"] == 0))
    rb2_zero = bool(np.all(g["rb2"] == 0))

    # --- bf16 weight pack ---
    wbp = np.zeros((128, WB_W), f32)

    def put(name, arr, row0=0):
        c0, c1 = WB_COLS[name]
        r, c = arr.shape
        assert c == c1 - c0, name
        wbp[row0:row0 + r, c0:c1] = arr

    put("kW1", g["kW1"])                       # rows 0-63 (64-67 stay zero)
    put("vW1", g["vW1"])                       # rows 0-67
    put("kW2a", g["kW2"][0:128]); put("kW2b", g["kW2"][128:256])
    put("vW2a", g["vW2"][0:128]); put("vW2b", g["vW2"][128:256])
    put("kW3a", g["kW3"][0:128]); put("kW3b", g["kW3"][128:256])
    put("vW3a", g["vW3"][0:128]); put("vW3b", g["vW3"][128:256])
    put("rW2a", g["rW2"][0:128]); put("rW2b", g["rW2"][128:256])
    put("rW1", g["rW1"])                       # rows 0-63
    put("rb2", g["rb2"].reshape(1, DOUT))

    # --- f32 pack A (per-feature biases; sw/mk filled per core) ---
    fa_shared = np.zeros((128, FA_W), f32)
    for nm in ("kb1", "vb1", "kb2", "vb2", "rb1"):
        c0, c1 = FA_COLS[nm]
        fa_shared[:, c0:c1] = g[nm].reshape(2, 128).T

    # --- f32 pack B ---
    fbp = np.zeros((64, FB_W), f32)

    def putb(name, arr, row0=0):
        c0, c1 = FB_COLS[name]
        r, c = arr.shape
        assert c == c1 - c0, name
        fbp[row0:row0 + r, c0:c1] = arr

    putb("qT", g["query_tokens"][0].T)
    putb("aW2", g["aW2"])
    putb("aW1", g["aW1"])
    putb("ab1", g["ab1"].reshape(AH, 1))
    putb("kb3", g["kb3"].reshape(DK, 1))
    putb("vb3", g["vb3"].reshape(DV, 1))
    putb("Bd", g["B_dirs"] * ADAPT_SCALE)      # rows 0-3
    putb("ab2", g["ab2"].reshape(1, P * R))

    wb16 = wbp.astype(BF16)
    in_maps = []
    for b in range(B):
        x_fm = x_enc[b].reshape(128, NT, DX).transpose(2, 1, 0).reshape(DX, N)
        u_fm = u[b].reshape(128, NT, DU).transpose(2, 1, 0).reshape(DU, N)
        xu_fm = np.concatenate([x_fm, u_fm], axis=0)
        fap = fa_shared.copy()
        fap[:, 0:64] = sw[b].reshape(128, NT)
        fap[:, 64:128] = mask[b].reshape(128, NT)
        in_maps.append({
            "xu_fm": np.ascontiguousarray(xu_fm).astype(BF16),
            "wb16": wb16,
            "f32a": fap,
            "f32b": fbp,
        })
    return in_maps, (biases_zero, vb3_zero, ab2_zero, rb2_zero)


def run(inputs, trace=False):
    _install_ntff_hook_shim()
    from concourse.bass_utils import run_bass_kernel_spmd

    in_maps, flags = _prep_maps(inputs)
    key = ("nc",) + flags
    if key not in _CACHE:
        _CACHE[key] = _build(*flags)
    nc = _CACHE[key]

    res = run_bass_kernel_spmd(nc, in_maps, core_ids=list(range(B)),
                               trace=trace)
    out = np.stack([res.results[b]["out"] for b in range(B)], axis=0)
    return out.astype(np.float32), res


def kernel(**inputs) -> np.ndarray:
    out, _ = run(inputs, trace=False)
    return out


# revision 17
# speedup vs baseline: 1.2961x; 1.2961x over previous
"""AdaptiveQuadratureHead Trainium2 kernel.

8-core SPMD, data-parallel over batch (B=8 -> one batch element per core).
No collectives. Host marshals inputs into feature-major bf16 layouts with a
virtual token order (physical token q*64+t -> tile t, partition q) so that
per-token scalars (quadrature weights) land as contiguous [128, 1] columns.

Math (per core, N=8192 tokens):
  K = gelu(gelu(xu @ W1k + kb1) @ kW2 + kb2) @ kW3 + kb3       (feature-major)
  V = gelu(gelu(xu @ W1v + vb1) @ vW2 + vb2) @ vW3 (token-major, h2-stationary)
  w = max(sensor_weights, 0) * mask ; denom = max(sum w, eps)
  c = (sum_t w*V)/denom + vb3
  a = tanh(gelu(c@aW1+ab1) @ aW2 + ab2) ; Q^T = qT + (0.1*B_dirs)^T a^T
  scoresT = K^T Q ; Phi = softplus(scoresT/8) = ln(exp(scoresT/8)+1)
  pooledT = (V*w | w)^T-accumulated Phi matmuls; /denom; +vb3 correction
  out = gelu(pooledT^T rW1 + rb1) @ rW2 + rb2
"""

import sys
import types
from contextlib import ExitStack

import numpy as np
import ml_dtypes

B, N = 8, 8192
DX, DU = 64, 4
P, DK, DV, DOUT = 128, 64, 64, 128
H, R, AH = 256, 4, 64
ADAPT_SCALE = 0.1
EPS = 1e-8
NT = 64          # token tiles of 128 (virtual order)
NCH = 16         # column chunks of 512
VW_W = 65        # V*w tile width (64 features + w column)
NG = 8           # score groups (8 tiles of 128 -> [128, 1024])
BF16 = ml_dtypes.bfloat16

# bf16 weight pack column offsets (rows 0-127)
WB_COLS = {
    "kW1": (0, 256), "vW1": (256, 512),
    "kW2a": (512, 768), "kW2b": (768, 1024),
    "vW2a": (1024, 1280), "vW2b": (1280, 1536),
    "kW3a": (1536, 1600), "kW3b": (1600, 1664),
    "vW3a": (1664, 1728), "vW3b": (1728, 1792),
    "rW2a": (1792, 1920), "rW2b": (1920, 2048),
    "rW1": (2048, 2304), "rb2": (2304, 2432),
}
WB_W = 2432
# f32 pack A [128, 138]
FA_COLS = {"sw": (0, 64), "mk": (64, 128), "kb1": (128, 130),
           "vb1": (130, 132), "kb2": (132, 134), "vb2": (134, 136),
           "rb1": (136, 138)}
FA_W = 138
# f32 pack B [64, 1283]
FB_COLS = {"qT": (0, 128), "aW2": (128, 640), "aW1": (640, 704),
           "ab1": (704, 705), "kb3": (705, 706), "vb3": (706, 707),
           "Bd": (707, 771), "ab2": (771, 1283)}
FB_W = 1283

_CACHE = {}


def _install_ntff_hook_shim():
    """This image's antenv lacks axon_hooks; provide it so
    run_bass_kernel_spmd(trace=True) can reach the ctypes NTFF hook."""
    if "antenv.axon_hooks" in sys.modules:
        return
    try:
        from trn_agent_boot.trn_boot import _ntff_profile_via_ctypes
        hook = _ntff_profile_via_ctypes("/opt/axon/libaxon_pjrt.so")
    except Exception:
        hook = None
    mod = types.ModuleType("antenv.axon_hooks")
    mod._hook = hook
    mod.get_axon_ntff_profile_hook = lambda: mod._hook
    mod.set_axon_ntff_profile_hook = lambda h: setattr(mod, "_hook", h)
    sys.modules["antenv.axon_hooks"] = mod
    try:
        import antenv
        antenv.axon_hooks = mod
    except Exception:
        pass


def _build(biases_zero: bool, vb3_zero: bool, ab2_zero: bool, rb2_zero: bool):
    import concourse.bass as bass
    import concourse.bacc as bacc
    import concourse.mybir as mybir
    import concourse.tile as tile
    from concourse.tile_rust import add_dep_helper

    AF = mybir.ActivationFunctionType
    OP = mybir.AluOpType
    AX = mybir.AxisListType
    f32 = mybir.dt.float32
    bf16 = mybir.dt.bfloat16

    nc = bacc.Bacc(None, target_bir_lowering=False)

    def din(name, shape, dt):
        return nc.declare_dram_parameter(name, list(shape), dt, isOutput=False)

    xu_d = din("xu_fm", (68, N), bf16)
    wb_d = din("wb16", (128, WB_W), bf16)
    fa_d = din("f32a", (128, FA_W), f32)
    fb_d = din("f32b", (64, FB_W), f32)
    out_d = nc.declare_dram_parameter("out", [P, DOUT], f32, isOutput=True)

    with tile.TileContext(nc) as tc, ExitStack() as ctx:
        const = ctx.enter_context(tc.tile_pool(name="const", bufs=1))
        xu_pool = ctx.enter_context(tc.tile_pool(name="xu", bufs=3))
        h_pool = ctx.enter_context(tc.tile_pool(name="hsb", bufs=6))
        phi_pool = ctx.enter_context(tc.tile_pool(name="phi", bufs=2))
        es_pool = ctx.enter_context(tc.tile_pool(name="es", bufs=NG))
        ps_big = ctx.enter_context(
            tc.tile_pool(name="psb", bufs=3, space=bass.MemorySpace.PSUM))
        ps_sm = ctx.enter_context(
            tc.tile_pool(name="pss", bufs=2, space=bass.MemorySpace.PSUM))

        # ---- packed constants: few big DMAs, spread across dispatchers ----
        wb_t = const.tile([128, WB_W], bf16, tag="wb")
        nc.gpsimd.dma_start(wb_t[:, 0:256], wb_d[:, 0:256])
        nc.gpsimd.dma_start(wb_t[:, 256:512], wb_d[:, 256:512])
        for c0, c1 in ((512, 1024), (1024, 1536), (1536, 2048),
                       (2048, WB_W)):
            nc.gpsimd.dma_start(wb_t[:, c0:c1], wb_d[:, c0:c1])
        fa_t = const.tile([128, FA_W], f32, tag="fa")
        nc.gpsimd.dma_start(fa_t[:], fa_d[:])
        fb_t = const.tile([64, FB_W], f32, tag="fb")
        nc.gpsimd.dma_start(fb_t[:], fb_d[:])

        def wb(name, rows=128):
            c0, c1 = WB_COLS[name]
            return wb_t[0:rows, c0:c1]

        def fa(name):
            c0, c1 = FA_COLS[name]
            return fa_t[:, c0:c1]

        def fb(name, rows=64):
            c0, c1 = FB_COLS[name]
            return fb_t[0:rows, c0:c1]

        K_fm = const.tile([DK, N], bf16, tag="K_fm")
        Vw_all = const.tile([128, NT * VW_W], bf16, tag="Vw")
        Vacc = const.tile([128, 4 * DV], f32, tag="Vacc")
        w_t = const.tile([128, NT], f32, tag="w")
        ones_col = const.tile([128, 1], f32, tag="ones_c")
        ones_r64 = const.tile([1, 64], f32, tag="ones_r")
        ones_rP = const.tile([1, P], bf16, tag="ones_p")

        nc.gpsimd.memset(Vacc[:], 0.0)
        nc.vector.memset(ones_col[:], 1.0)
        nc.vector.memset(ones_r64[:], 1.0)
        nc.vector.memset(ones_rP[:], 1.0)

        # quadrature weights: w = max(sw, 0) * mask
        nc.vector.tensor_scalar(w_t[:], fa("sw"), 0.0, None, OP.max)
        nc.vector.tensor_tensor(w_t[:], w_t[:], fa("mk"), op=OP.mult)

        Vw_v = Vw_all[:].rearrange("p (t c) -> p t c", c=VW_W)

        # ================= PASS A: token MLPs (K+V fused) =================
        for j in range(NCH):
            cs = slice(j * 512, (j + 1) * 512)
            xu = xu_pool.tile([68, 512], bf16, tag="xu")
            nc.sync.dma_start(xu[:], xu_d[:, cs])

            # L1
            h1K_ps = ps_big.tile([128, 1024], f32, tag="ps")
            nc.tensor.matmul(h1K_ps[:, 0:512], wb("kW1", 68)[:, 0:128], xu[:])
            nc.tensor.matmul(h1K_ps[:, 512:1024], wb("kW1", 68)[:, 128:256],
                             xu[:])
            h1V_ps = ps_big.tile([128, 1024], f32, tag="ps")
            nc.tensor.matmul(h1V_ps[:, 0:512], wb("vW1", 68)[:, 0:128], xu[:])
            nc.tensor.matmul(h1V_ps[:, 512:1024], wb("vW1", 68)[:, 128:256],
                             xu[:])

            h1K = h_pool.tile([128, 1024], bf16, tag="h")
            h1V = h_pool.tile([128, 1024], bf16, tag="h")
            if biases_zero:
                nc.scalar.activation(h1K[:], h1K_ps[:], AF.Gelu)
                nc.scalar.activation(h1V[:], h1V_ps[:], AF.Gelu)
            else:
                for ho in range(2):
                    hs = slice(ho * 512, (ho + 1) * 512)
                    nc.scalar.activation(h1K[:, hs], h1K_ps[:, hs], AF.Gelu,
                                         bias=fa("kb1")[:, ho:ho + 1])
                    nc.scalar.activation(h1V[:, hs], h1V_ps[:, hs], AF.Gelu,
                                         bias=fa("vb1")[:, ho:ho + 1])

            # L2
            h2K_ps = ps_big.tile([128, 1024], f32, tag="ps")
            h2V_ps = ps_big.tile([128, 1024], f32, tag="ps")
            for ho in range(2):
                hs = slice(ho * 512, (ho + 1) * 512)
                wsl = slice(ho * 128, (ho + 1) * 128)
                nc.tensor.matmul(h2K_ps[:, hs], wb("kW2a")[:, wsl],
                                 h1K[:, 0:512], start=True, stop=False)
                nc.tensor.matmul(h2K_ps[:, hs], wb("kW2b")[:, wsl],
                                 h1K[:, 512:1024], start=False, stop=True)
                nc.tensor.matmul(h2V_ps[:, hs], wb("vW2a")[:, wsl],
                                 h1V[:, 0:512], start=True, stop=False)
                nc.tensor.matmul(h2V_ps[:, hs], wb("vW2b")[:, wsl],
                                 h1V[:, 512:1024], start=False, stop=True)

            h2K = h_pool.tile([128, 1024], bf16, tag="h")
            h2V = h_pool.tile([128, 1024], bf16, tag="h")
            if biases_zero:
                nc.scalar.activation(h2K[:], h2K_ps[:], AF.Gelu)
                nc.scalar.activation(h2V[:], h2V_ps[:], AF.Gelu)
            else:
                for ho in range(2):
                    hs = slice(ho * 512, (ho + 1) * 512)
                    nc.scalar.activation(h2K[:, hs], h2K_ps[:, hs], AF.Gelu,
                                         bias=fa("kb2")[:, ho:ho + 1])
                    nc.scalar.activation(h2V[:, hs], h2V_ps[:, hs], AF.Gelu,
                                         bias=fa("vb2")[:, ho:ho + 1])

            # L3 K-net: feature-major K
            k_ps = ps_sm.tile([DK, 512], f32, tag="ps")
            nc.tensor.matmul(k_ps[:], wb("kW3a"), h2K[:, 0:512],
                             start=True, stop=False)
            nc.tensor.matmul(k_ps[:], wb("kW3b"), h2K[:, 512:1024],
                             start=False, stop=True)
            nc.vector.tensor_scalar_add(K_fm[:, cs], k_ps[:], fb("kb3"))

            # L3 V-net: token-major V tiles [128tok, 64]
            v_ps = ps_sm.tile([128, 4 * DV], f32, tag="ps")
            for st in range(4):
                vs = slice(st * DV, (st + 1) * DV)
                ts_a = slice(st * 128, (st + 1) * 128)
                ts_b = slice(512 + st * 128, 512 + (st + 1) * 128)
                nc.tensor.matmul(v_ps[:, vs], h2V[:, ts_a], wb("vW3a"),
                                 start=True, stop=False)
                nc.tensor.matmul(v_ps[:, vs], h2V[:, ts_b], wb("vW3b"),
                                 start=False, stop=True)

            tsl = slice(4 * j, 4 * j + 4)
            w_b = w_t[:, tsl].to_broadcast([128, 4, DV])
            v4 = v_ps[:].rearrange("p (t c) -> p t c", c=DV)
            nc.vector.tensor_tensor(Vw_v[:, tsl, 0:DV], v4, w_b, op=OP.mult)
            if not vb3_zero:
                nc.vector.tensor_copy(
                    Vw_v[:, tsl, DV:VW_W],
                    w_t[:, tsl].rearrange("p (t c) -> p t c", c=1))
            nc.vector.tensor_tensor(Vacc[:], Vacc[:], Vw_v[:, tsl, 0:DV],
                                    op=OP.add)

        # ================= PASS B: c, denom, adaptive query =================
        wsum = const.tile([128, 1], f32, tag="wsum")
        nc.vector.reduce_sum(wsum[:], w_t[:], axis=AX.X)
        den_ps = ps_sm.tile([1, 1], f32, tag="ps")
        nc.tensor.matmul(den_ps[:], wsum[:], ones_col[:])
        den_t = const.tile([1, 1], f32, tag="den")
        nc.vector.tensor_scalar(den_t[:], den_ps[:], EPS, None, OP.max)
        rec_t = const.tile([1, 1], f32, tag="rec")
        nc.vector.reciprocal(rec_t[:], den_t[:])
        recb_ps = ps_sm.tile([64, 1], f32, tag="ps")
        nc.tensor.matmul(recb_ps[:], ones_r64[:], rec_t[:])
        recb_t = const.tile([64, 1], f32, tag="recb")
        nc.vector.tensor_copy(recb_t[:], recb_ps[:])

        Vred = const.tile([128, DV], f32, tag="Vred")
        nc.vector.reduce_sum(
            Vred[:], Vacc[:].rearrange("p (t c) -> p c t", c=DV), axis=AX.X)
        c_ps = ps_sm.tile([DV, 1], f32, tag="ps")
        nc.tensor.matmul(c_ps[:], Vred[:], ones_col[:])

        # c = c_raw/denom + vb3   (max(denom,eps)=denom whenever w-sum > eps)
        c_t = const.tile([DV, 1], f32, tag="c")
        nc.vector.scalar_tensor_tensor(c_t[:], c_ps[:], recb_t[:], fb("vb3"),
                                       op0=OP.mult, op1=OP.add)

        # adaptive query: g = gelu(aW1.T c + ab1); a = tanh(aW2.T g + ab2)
        g_ps = ps_sm.tile([AH, 1], f32, tag="ps")
        nc.tensor.matmul(g_ps[:], fb("aW1"), c_t[:])
        g_t = const.tile([AH, 1], f32, tag="g")
        nc.scalar.activation(g_t[:], g_ps[:], AF.Gelu, bias=fb("ab1"))
        a_ps = ps_sm.tile([1, P * R], f32, tag="ps")
        nc.tensor.matmul(a_ps[:], g_t[:], fb("aW2"))
        av_t = const.tile([1, P * R], f32, tag="av")
        if ab2_zero:
            nc.scalar.activation(av_t[:], a_ps[:], AF.Tanh)
        else:
            a1_t = const.tile([1, P * R], f32, tag="a1")
            nc.vector.tensor_tensor(a1_t[:], a_ps[:], fb("ab2", 1), op=OP.add)
            nc.scalar.activation(av_t[:], a1_t[:], AF.Tanh)
        aT_t = const.tile([R, P], f32, tag="aT")
        av_v = av_t[0:1, :].rearrange("o (p r) -> o r p", r=R)
        for r, eng in enumerate((nc.sync, nc.scalar, nc.gpsimd, nc.sync)):
            eng.dma_start(aT_t[r:r + 1, :], av_v[:, r, :])
        qd_ps = ps_sm.tile([DK, P], f32, tag="ps")
        nc.tensor.matmul(qd_ps[:], fb("Bd", R), aT_t[:])
        QT_t = const.tile([DK, P], bf16, tag="QT")
        nc.vector.tensor_tensor(QT_t[:], qd_ps[:], fb("qT"), op=OP.add)

        # ================= PASS C: scores + pooled =================
        # softplus(s/sqrt(dk)) = ln(exp(s/8) + 1); scores scale ~0.1 so exp
        # cannot overflow. No native softplus table in this toolchain, and
        # the table chooser puts Exp and Ln in different sets, so run ALL
        # exps then ALL lns (ordering-pinned) to avoid table-load thrash.
        pool_m = DV if vb3_zero else VW_W
        pool_ps = ps_sm.tile([pool_m, P], f32, tag="ps")
        es_tiles = []
        last_exp = None
        for g in range(NG):
            sc_ps = ps_big.tile([128, 1024], f32, tag="ps")
            for s in range(8):
                t = g * 8 + s
                nc.tensor.matmul(sc_ps[:, s * 128:(s + 1) * 128],
                                 K_fm[:, t * 128:(t + 1) * 128], QT_t[:])
            es = es_pool.tile([128, 1024], f32, tag="es")
            last_exp = nc.scalar.activation(es[:], sc_ps[:], AF.Exp,
                                            scale=float(1.0 / np.sqrt(DK)))
            es_tiles.append(es)
        for g in range(NG):
            phi = phi_pool.tile([128, 1024], bf16, tag="phi")
            ln_i = nc.scalar.activation(phi[:], es_tiles[g][:], AF.Ln,
                                        bias=1.0)
            add_dep_helper(ln_i.ins, last_exp.ins, sync=False,
                           reason="batch act-table sets: all exps before lns")
            for s in range(8):
                t = g * 8 + s
                nc.tensor.matmul(pool_ps[:], Vw_v[:, t, 0:pool_m],
                                 phi[:, s * 128:(s + 1) * 128],
                                 start=(t == 0), stop=(t == NT - 1))

        # ================= PASS D: normalize + rho MLP =================
        poolb_t = const.tile([DV, P], bf16, tag="poolb")
        if vb3_zero:
            nc.vector.tensor_scalar_mul(poolb_t[:], pool_ps[0:DV, :],
                                        recb_t[:])
        else:
            swp_t = const.tile([1, P], f32, tag="swp")
            nc.vector.tensor_scalar_mul(swp_t[:], pool_ps[DV:VW_W, :],
                                        rec_t[:])
            swpb_ps = ps_sm.tile([DV, P], f32, tag="ps")
            nc.tensor.matmul(swpb_ps[:], ones_r64[:], swp_t[:])
            pooln_t = const.tile([DV, P], f32, tag="pooln")
            nc.vector.tensor_scalar_mul(pooln_t[:], pool_ps[0:DV, :],
                                        recb_t[:])
            nc.vector.scalar_tensor_tensor(poolb_t[:], swpb_ps[:], fb("vb3"),
                                           pooln_t[:], op0=OP.mult,
                                           op1=OP.add)

        # rho L1: hr = gelu(rW1.T pooledT + rb1)
        hr_sb = []
        for hc in range(2):
            hr_ps = ps_sm.tile([128, P], f32, tag="ps")
            nc.tensor.matmul(hr_ps[:],
                             wb("rW1", 64)[:, hc * 128:(hc + 1) * 128],
                             poolb_t[:])
            hr = const.tile([128, P], bf16, tag=f"hr{hc}")
            nc.scalar.activation(hr[:], hr_ps[:], AF.Gelu,
                                 bias=fa("rb1")[:, hc:hc + 1])
            hr_sb.append(hr)

        # rho L2: out = hr.T rW2 + rb2  (bias via rank-1 ones matmul)
        o_ps = ps_sm.tile([P, DOUT], f32, tag="ps")
        nc.tensor.matmul(o_ps[:], hr_sb[0][:], wb("rW2a"),
                         start=True, stop=False)
        nc.tensor.matmul(o_ps[:], hr_sb[1][:], wb("rW2b"),
                         start=False, stop=rb2_zero)
        if not rb2_zero:
            nc.tensor.matmul(o_ps[:], ones_rP[:], wb("rb2", 1),
                             start=False, stop=True)
        o_sb = const.tile([P, DOUT], f32, tag="osb")
        nc.vector.tensor_copy(o_sb[:], o_ps[:])
        nc.sync.dma_start(out_d[:], o_sb[:])

    nc.compile()
    return nc


def _prep_maps(inputs):
    f32 = np.float32
    x_enc = np.asarray(inputs["x_enc"], f32)
    u = np.asarray(inputs["u"], f32)
    mask = np.asarray(inputs["sensor_mask"]).astype(f32)
    sw = np.asarray(inputs["sensor_weights"], f32)

    g = {k: np.asarray(inputs[k], f32) for k in
         ("kW1", "kb1", "kW2", "kb2", "kW3", "kb3",
          "vW1", "vb1", "vW2", "vb2", "vW3", "vb3",
          "query_tokens", "B_dirs", "aW1", "ab1", "aW2", "ab2",
          "rW1", "rb1", "rW2", "rb2")}

    biases_zero = bool(all(np.all(g[k] == 0)
                           for k in ("kb1", "vb1", "kb2", "vb2")))
    vb3_zero = bool(np.all(g["vb3"] == 0))
    ab2_zero = bool(np.all(g["ab2"] == 0))
    rb2_zero = bool(np.all(g["rb2"] == 0))

    # --- bf16 weight pack ---
    wbp = np.zeros((128, WB_W), f32)

    def put(name, arr, row0=0):
        c0, c1 = WB_COLS[name]
        r, c = arr.shape
        assert c == c1 - c0, name
        wbp[row0:row0 + r, c0:c1] = arr

    put("kW1", g["kW1"])                       # rows 0-63 (64-67 stay zero)
    put("vW1", g["vW1"])                       # rows 0-67
    put("kW2a", g["kW2"][0:128]); put("kW2b", g["kW2"][128:256])
    put("vW2a", g["vW2"][0:128]); put("vW2b", g["vW2"][128:256])
    put("kW3a", g["kW3"][0:128]); put("kW3b", g["kW3"][128:256])
    put("vW3a", g["vW3"][0:128]); put("vW3b", g["vW3"][128:256])
    put("rW2a", g["rW2"][0:128]); put("rW2b", g["rW2"][128:256])
    put("rW1", g["rW1"])                       # rows 0-63
    put("rb2", g["rb2"].reshape(1, DOUT))

    # --- f32 pack A (per-feature biases; sw/mk filled per core) ---
    fa_shared = np.zeros((128, FA_W), f32)
    for nm in ("kb1", "vb1", "kb2", "vb2", "rb1"):
        c0, c1 = FA_COLS[nm]
        fa_shared[:, c0:c1] = g[nm].reshape(2, 128).T

    # --- f32 pack B ---
    fbp = np.zeros((64, FB_W), f32)

    def putb(name, arr, row0=0):
        c0, c1 = FB_COLS[name]
        r, c = arr.shape
        assert c == c1 - c0, name
        fbp[row0:row0 + r, c0:c1] = arr

    putb("qT", g["query_tokens"][0].T)
    putb("aW2", g["aW2"])
    putb("aW1", g["aW1"])
    putb("ab1", g["ab1"].reshape(AH, 1))
    putb("kb3", g["kb3"].reshape(DK, 1))
    putb("vb3", g["vb3"].reshape(DV, 1))
    putb("Bd", g["B_dirs"] * ADAPT_SCALE)      # rows 0-3
    putb("ab2", g["ab2"].reshape(1, P * R))

    wb16 = wbp.astype(BF16)
    in_maps = []
    for b in range(B):
        x_fm = x_enc[b].reshape(128, NT, DX).transpose(2, 1, 0).reshape(DX, N)
        u_fm = u[b].reshape(128, NT, DU).transpose(2, 1, 0).reshape(DU, N)
        xu_fm = np.concatenate([x_fm, u_fm], axis=0)
        fap = fa_shared.copy()
        fap[:, 0:64] = sw[b].reshape(128, NT)
        fap[:, 64:128] = mask[b].reshape(128, NT)
        in_maps.append({
            "xu_fm": np.ascontiguousarray(xu_fm).astype(BF16),
            "wb16": wb16,
            "f32a": fap,
            "f32b": fbp,
        })
    return in_maps, (biases_zero, vb3_zero, ab2_zero, rb2_zero)


def run(inputs, trace=False):
    _install_ntff_hook_shim()
    from concourse.bass_utils import run_bass_kernel_spmd

    in_maps, flags = _prep_maps(inputs)
    key = ("nc",) + flags
    if key not in _CACHE:
        _CACHE[key] = _build(*flags)
    nc = _CACHE[key]

    res = run_bass_kernel_spmd(nc, in_maps, core_ids=list(range(B)),
                               trace=trace)
    out = np.stack([res.results[b]["out"] for b in range(B)], axis=0)
    return out.astype(np.float32), res


def kernel(**inputs) -> np.ndarray:
    out, _ = run(inputs, trace=False)
    return out
